# revision 3
# baseline (speedup 1.0000x reference)
"""Trainium2 Bass kernel for a single attention head (B=4, S=2048, D=4096, DH=128).

Sharding: 8 cores = (batch b, half h), pair (2b, 2b+1) shares batch b.
Core (b,h) owns the INTERLEAVED tile set {t : t % 2 == h} (8 tiles of 128
positions) -- both its q rows AND its K/V key chunks. Each core projects
Q, K, V only for its OWN 1024 columns of x (half the work of replicating
K/V), then the pair exchanges K/V halves with a pairwise AllGather.

Rank-symmetric layout (same NEFF on every core): K^T / V go through a
DRAM bounce + AllGather and are read back RANK-indexed (rank0 = even
tiles, rank1 = odd tiles), so no access pattern depends on h. The
causal structure relative to rank0/rank1 is baked into the host-provided
mask block (triangle / zeros / -1e9), exactly like the baseline's
own/other mask trick.

Phases:
  1: two column-stages (own cols 0:512, 512:1024). Per stage, stream x
     chunks and accumulate K (+Q in stage A) and V. Stage drains feed a
     bounce buffer; AllGather per stage; readback fills kt_sb/v_sb both
     rank halves. Q of stage B is deferred until after stage B's K/V so
     the second gather starts as early as possible.
  2: per local q-tile j (global 2j+h): logits over rank0 range
     [0:(j+1)*128] and rank1 range [1024:1024+(j+1)*128], mask on the
     last chunk of each range -> rowmax -> exp (accum rowsum) -> W^T via
     DMA transpose -> PV -> scale -> out. Small tiles (gather-A-only)
     run first to cover gather B's latency; large tiles largest-first.
"""

import numpy as np
import ml_dtypes

import concourse.bass as bass
import concourse.tile as tile
from concourse import bacc, mybir
from concourse.bass_utils import run_bass_kernel_spmd

B, S, D, DH = 4, 2048, 4096, 128
SQ = S // 2          # own q rows / own key cols per core
ST = 512             # columns per stage
N_CORES = 8
D_CH = D // 128      # 32 contraction chunks
NT = 8               # local q tiles (slots)
PAIRS = [[0, 1], [2, 3], [4, 5], [6, 7]]

BF16 = mybir.dt.bfloat16
F32 = mybir.dt.float32


def build_nc():
    nc = bacc.Bacc(None)

    # x own columns, stage-major: xT[st*128+p, i, s] = x[b, col(st*512+s), i*128+p]
    xT = nc.dram_tensor("xT", [2 * 128, D_CH, ST], BF16, kind="ExternalInput")
    mask = nc.dram_tensor("mask", [128, 256], BF16, kind="ExternalInput")
    # weights pre-tiled on host: w[p, i, m] = W[m, i*128+p]
    wqT = nc.dram_tensor("wqT", [128, D_CH, DH], BF16, kind="ExternalInput")
    wkT = nc.dram_tensor("wkT", [128, D_CH, DH], BF16, kind="ExternalInput")
    wvT = nc.dram_tensor("wvT", [128, D_CH, DH], BF16, kind="ExternalInput")
    bq = nc.dram_tensor("bq", [DH, 1], F32, kind="ExternalInput")
    bk = nc.dram_tensor("bk", [DH, 1], F32, kind="ExternalInput")
    bv = nc.dram_tensor("bv", [DH, 1], F32, kind="ExternalInput")
    out = nc.dram_tensor("out", [SQ, DH], BF16, kind="ExternalOutput")
    # comm bounce: per stage [128, 1024] = [K^T 512 | V(pos-major) 512]
    cc_in = nc.dram_tensor("cc_in", [2 * 128, 2 * ST], BF16, kind="Internal")
    cc_out = nc.dram_tensor("cc_out", [2 * 256, 2 * ST], BF16, kind="Internal")

    with tile.TileContext(nc) as tc:
        with (
            tc.tile_pool(name="weights", bufs=1) as wpool,
            tc.tile_pool(name="persist", bufs=1) as persist,
        ):
            w_sb = {}
            for name in ("q", "k", "v"):
                w_sb[name] = wpool.tile([128, D_CH, DH], BF16, tag=f"w{name}",
                                        name=f"w{name}")
            W_EXT = {"q": wqT, "k": wkT, "v": wvT}

            def w_slice(g):
                ss = np.s_[:, g * 4:(g + 1) * 4, :]
                for name in ("k", "q", "v"):
                    nc.sync.dma_start(out=w_sb[name][ss], in_=W_EXT[name][ss])
            b_sb = {}
            for name, ext in (("q", bq), ("k", bk), ("v", bv)):
                t = wpool.tile([DH, 1], F32, tag=f"b{name}")
                nc.sync.dma_start(out=t[:], in_=ext[:])
                b_sb[name] = t
            w_slice(0)
            w_slice(1)
            mk = persist.tile([128, 256], BF16, tag="mk")
            nc.gpsimd.dma_start(out=mk[:], in_=mask[:])

            # rank-indexed: rank0 keys at [0:1024], rank1 at [1024:2048]
            kt_sb = persist.tile([DH, S], BF16, tag="kt")
            v_sb = persist.tile([128, 2 * NT, DH], BF16, tag="v")
            qt_sb = persist.tile([DH, SQ], BF16, tag="qt")   # own Q^T
            x_st = [persist.tile([128, D_CH, ST], BF16, tag=f"x{st}",
                                 name=f"x{st}") for st in (0, 1)]
            kst = [persist.tile([DH, ST], BF16, tag=f"kst{st}", name=f"kst{st}")
                   for st in (0, 1)]
            vst = [persist.tile([DH, ST], BF16, tag=f"vst{st}", name=f"vst{st}")
                   for st in (0, 1)]
            vtr = [persist.tile([128, 4, DH], BF16, tag=f"vtr{st}", name=f"vtr{st}")
                   for st in (0, 1)]

            # --- phase 1: projections in two column-stages + pair AllGather ---
            with tc.tile_pool(name="ppsum1", bufs=1, space="PSUM") as pp1:
                acc = {}
                for tag in ("pk0", "pq0", "pv0", "pk1", "pv1", "pq1"):
                    acc[tag] = pp1.tile([DH, ST], F32, tag=tag, name=tag)

                def x_piece(st, i0, n):
                    nc.sync.dma_start(out=x_st[st][:, i0:i0 + n, :],
                                      in_=xT[st * 128:(st + 1) * 128, i0:i0 + n, :])

                def stage_mms(st, with_q):
                    for i in range(D_CH):
                        if st == 0 and i % 4 == 0 and 2 + i // 4 < 8:
                            w_slice(2 + i // 4)
                        if i % 4 == 0:
                            x_piece(st, i, 4)
                        stt = dict(start=(i == 0), stop=(i == D_CH - 1))
                        nc.tensor.matmul(acc[f"pk{st}"][:], lhsT=w_sb["k"][:, i, :],
                                         rhs=x_st[st][:, i, :], **stt)
                        if with_q:
                            nc.tensor.matmul(acc[f"pq{st}"][:], lhsT=w_sb["q"][:, i, :],
                                             rhs=x_st[st][:, i, :], **stt)
                        nc.tensor.matmul(acc[f"pv{st}"][:], lhsT=w_sb["v"][:, i, :],
                                         rhs=x_st[st][:, i, :], **stt)

                def stage_comm(st):
                    o = st * ST
                    nc.vector.tensor_scalar_add(kst[st][:], acc[f"pk{st}"][:],
                                                b_sb["k"][:])
                    nc.vector.tensor_scalar_add(vst[st][:], acc[f"pv{st}"][:],
                                                b_sb["v"][:])
                    nc.sync.dma_start_transpose(out=vtr[st][:], in_=vst[st][:])
                    nc.gpsimd.dma_start(out=cc_in[st * 128:(st + 1) * 128, 0:ST],
                                        in_=kst[st][:])
                    nc.gpsimd.dma_start(out=cc_in[st * 128:(st + 1) * 128, ST:2 * ST],
                                        in_=vtr[st][:])
                    nc.gpsimd.collective_compute(
                        "AllGather", mybir.AluOpType.bypass,
                        replica_groups=PAIRS,
                        ins=[cc_in[st * 128:(st + 1) * 128, :].opt()],
                        outs=[cc_out[st * 256:(st + 1) * 256, :].opt()],
                    )
                    r0 = st * 256
                    r1 = st * 256 + 128
                    nc.sync.dma_start(out=kt_sb[:, o:o + ST],
                                      in_=cc_out[r0:r0 + 128, 0:ST])
                    nc.sync.dma_start(out=kt_sb[:, SQ + o:SQ + o + ST],
                                      in_=cc_out[r1:r1 + 128, 0:ST])
                    nc.sync.dma_start(out=v_sb[:, st * 4:st * 4 + 4, :],
                                      in_=cc_out[r0:r0 + 128, ST:2 * ST])
                    nc.sync.dma_start(out=v_sb[:, NT + st * 4:NT + st * 4 + 4, :],
                                      in_=cc_out[r1:r1 + 128, ST:2 * ST])

                # stage A: K, Q, V interleaved per chunk
                stage_mms(0, with_q=True)
                nc.vector.tensor_scalar_add(qt_sb[:, 0:ST], acc["pq0"][:],
                                            b_sb["q"][:])
                stage_comm(0)
                # stage B: K, V only (Q deferred so gather B starts early)
                stage_mms(1, with_q=False)
                stage_comm(1)
                # Q of stage B
                for i in range(D_CH):
                    nc.tensor.matmul(acc["pq1"][:], lhsT=w_sb["q"][:, i, :],
                                     rhs=x_st[1][:, i, :],
                                     start=(i == 0), stop=(i == D_CH - 1))
                nc.vector.tensor_scalar_add(qt_sb[:, ST:2 * ST], acc["pq1"][:],
                                            b_sb["q"][:])

            # --- phase 2: per-tile softmax + PV (baseline machinery) ---
            with (
                tc.tile_pool(name="lg_psum", bufs=4, space="PSUM") as lg,
                tc.tile_pool(name="o_psum", bufs=2, space="PSUM") as opool,
                tc.tile_pool(name="lm_sb", bufs=1) as lmpool,
                tc.tile_pool(name="wt_sb", bufs=1) as wtpool,
                tc.tile_pool(name="stats", bufs=12) as stat,
                tc.tile_pool(name="out_sb", bufs=2) as ospool,
            ):
                pv_args = {}

                def softmax_stage(j):
                    e = j + 1            # chunks per rank range
                    w = e * 128          # cols per rank range
                    qsl = np.s_[:, j * 128:(j + 1) * 128]

                    # Only the LAST chunk of each rank range carries mask
                    # values (host bakes h into mk): rank0 last = triangle
                    # (h=0) or all-0 (h=1); rank1 last = all--1e9 (h=0) or
                    # triangle (h=1). Other chunks are pure past: plain
                    # psum f32 -> bf16 rounding.
                    lmt = lmpool.tile([128, 2 * w], BF16, tag=f"lm{j}")
                    for base, off, mcol in ((0, 0, 0), (SQ, w, 128)):
                        for g0 in range(0, w, 512):
                            gw = min(512, w - g0)
                            pg = lg.tile([128, 512], F32, tag="pg")
                            nc.tensor.matmul(pg[:, :gw], lhsT=qt_sb[qsl],
                                             rhs=kt_sb[:, base + g0:base + g0 + gw],
                                             start=True, stop=True)
                            last = g0 + gw == w
                            cp = gw - 128 if last else gw
                            if cp:
                                nc.vector.tensor_copy(
                                    lmt[:, off + g0:off + g0 + cp], pg[:, :cp])
                            if last:
                                nc.vector.tensor_add(
                                    lmt[:, off + g0 + cp:off + g0 + gw],
                                    pg[:, cp:gw],
                                    mk[:, mcol:mcol + 128])
                    negmax = stat.tile([128, 1], F32, tag="negmax")
                    nc.vector.reduce_max(out=negmax[:], in_=lmt[:, :2 * w],
                                         axis=mybir.AxisListType.X, negate=True)
                    w_t = lmpool.tile([128, 2 * w], BF16, tag=f"w{j}")
                    wt_t = wtpool.tile([128, 2 * e, 128], BF16, tag=f"wt{j}")
                    sumexp = stat.tile([128, 1], F32, tag="sumexp")
                    nc.scalar.activation(
                        out=w_t[:, :2 * w], in_=lmt[:, :2 * w],
                        func=mybir.ActivationFunctionType.Exp,
                        bias=negmax[:], scale=1.0, accum_out=sumexp[:])
                    nc.sync.dma_start_transpose(out=wt_t[:, :2 * e, :],
                                                in_=w_t[:, :2 * w])
                    pv_args[j] = (wt_t, sumexp, e)

                def pv_stage(j):
                    wt_t, sumexp, e = pv_args.pop(j)
                    rsum = stat.tile([128, 1], F32, tag="rsum")
                    nc.vector.reciprocal(rsum[:], sumexp[:])
                    po = opool.tile([128, DH], F32, tag="po")
                    for c in range(2 * e):
                        vc = c if c < e else NT + (c - e)
                        nc.tensor.matmul(po[:], lhsT=wt_t[:, c, :], rhs=v_sb[:, vc, :],
                                         start=(c == 0), stop=(c == 2 * e - 1))
                    o_sb = ospool.tile([128, DH], BF16, tag="o")
                    nc.vector.tensor_scalar_mul(o_sb[:], po[:], rsum[:])
                    nc.sync.dma_start(out=out[j * 128:(j + 1) * 128, :], in_=o_sb[:])

                # small tiles (need only gather A) first -- they cover
                # gather B's latency; then large tiles largest-first so the
                # last softmax chains hide under other tiles' PE work
                softmax_stage(3)
                softmax_stage(2)
                softmax_stage(1)
                softmax_stage(0)
                pv_stage(3)
                pv_stage(2)
                pv_stage(1)
                pv_stage(0)
                softmax_stage(7)
                softmax_stage(6)
                softmax_stage(5)
                pv_stage(7)
                softmax_stage(4)
                pv_stage(6)
                pv_stage(5)
                pv_stage(4)

    nc.finalize()
    return nc


def shard_inputs(x, attn_mask, Wq, bq, Wk, bk, Wv, bv):
    """Host-side shard prep. Returns in_maps for cores 0..7."""
    bf = ml_dtypes.bfloat16
    xb = np.asarray(x).astype(bf)                   # cast first, like the reference
    mask_f = np.asarray(attn_mask)

    def tile_w(W):
        WT = np.asarray(W).astype(bf).T.reshape(D_CH, 128, DH)
        return np.ascontiguousarray(WT.transpose(1, 0, 2))

    wqt, wkt, wvt = tile_w(Wq), tile_w(Wk), tile_w(Wv)
    bqc = np.asarray(bq).astype(bf).astype(np.float32).reshape(DH, 1)
    bkc = np.asarray(bk).astype(bf).astype(np.float32).reshape(DH, 1)
    bvc = np.asarray(bv).astype(bf).astype(np.float32).reshape(DH, 1)

    tri = mask_f[:128, :128].astype(bf)       # causal triangle (0/-1e9)
    zeros = np.zeros((128, 128), dtype=bf)
    neg = np.full((128, 128), -1e9, dtype=np.float32).astype(bf)

    in_maps = []
    for c in range(N_CORES):
        b, h = divmod(c, 2)
        own = np.concatenate([np.arange(t * 128, (t + 1) * 128)
                              for t in range(h, 16, 2)])
        xcols = xb[b][own]                            # [1024, D]
        # [st, s, i, p] -> [st, p, i, s], flatten stage into rows
        xa = np.ascontiguousarray(
            xcols.reshape(2, ST, D_CH, 128).transpose(0, 3, 2, 1)
        ).reshape(2 * 128, D_CH, ST)
        if h == 0:
            msk = np.concatenate([tri, neg], axis=1)       # rank0 diag, rank1 future
        else:
            msk = np.concatenate([zeros, tri], axis=1)     # rank0 past, rank1 diag
        in_maps.append({
            "xT": xa, "mask": np.ascontiguousarray(msk),
            "wqT": wqt, "wkT": wkt, "wvT": wvt,
            "bq": bqc, "bk": bkc, "bv": bvc,
        })
    return in_maps


_NC_CACHE = {}


def kernel(x, attn_mask, Wq, bq, Wk, bk, Wv, bv):
    if "nc" not in _NC_CACHE:
        _NC_CACHE["nc"] = build_nc()
    nc = _NC_CACHE["nc"]
    in_maps = shard_inputs(x, attn_mask, Wq, bq, Wk, bk, Wv, bv)
    res = run_bass_kernel_spmd(nc, in_maps, list(range(N_CORES)))
    out = np.empty((B, S, DH), dtype=ml_dtypes.bfloat16)
    for c in range(N_CORES):
        b, h = divmod(c, 2)
        for j in range(NT):
            t = 2 * j + h
            out[b, t * 128:(t + 1) * 128, :] = res.results[c]["out"][j * 128:(j + 1) * 128]
    return out


# revision 8
# speedup vs baseline: 1.0960x; 1.0960x over previous
"""Trainium2 Bass kernel for a single attention head (B=4, S=2048, D=4096, DH=128).

Sharding: 8 cores = (batch b, half h), pair (2b, 2b+1) shares batch b.
Core (b,h) owns the INTERLEAVED tile set {t : t % 2 == h} (8 tiles of 128
positions) -- both its q rows AND its K/V key chunks. Each core projects
Q, K, V only for its OWN 1024 columns of x (half the work of replicating
K/V), then the pair exchanges K/V halves with a pairwise AllGather.

Rank-symmetric layout (same NEFF on every core): K^T / V go through a
DRAM bounce + AllGather and are read back RANK-indexed (rank0 = even
tiles, rank1 = odd tiles), so no access pattern depends on h. The
causal structure relative to rank0/rank1 is baked into the host-provided
mask block (triangle / zeros / -1e9), exactly like the baseline's
own/other mask trick.

Phases:
  1: two column-stages (own cols 0:512, 512:1024). Per stage, stream x
     chunks and accumulate K (+Q in stage A) and V. Stage drains feed a
     bounce buffer; AllGather per stage; readback fills kt_sb/v_sb both
     rank halves. Q of stage B is deferred until after stage B's K/V so
     the second gather starts as early as possible.
  2: per local q-tile j (global 2j+h): logits over rank0 range
     [0:(j+1)*128] and rank1 range [1024:1024+(j+1)*128], mask on the
     last chunk of each range -> rowmax -> exp (accum rowsum) -> W^T via
     DMA transpose -> PV -> scale -> out. Small tiles (gather-A-only)
     run first to cover gather B's latency; large tiles largest-first.
"""

import numpy as np
import ml_dtypes

import concourse.bass as bass
import concourse.tile as tile
from concourse import bacc, mybir
from concourse.bass_utils import run_bass_kernel_spmd

B, S, D, DH = 4, 2048, 4096, 128
SQ = S // 2          # own q rows / own key cols per core
ST = 512             # columns per stage
N_CORES = 8
D_CH = D // 128      # 32 contraction chunks
NT = 8               # local q tiles (slots)
PAIRS = [[0, 1], [2, 3], [4, 5], [6, 7]]

BF16 = mybir.dt.bfloat16
F32 = mybir.dt.float32


def build_nc():
    nc = bacc.Bacc(None)

    # x own columns, stage-major: xT[st*128+p, i, s] = x[b, col(st*512+s), i*128+p]
    xT = nc.dram_tensor("xT", [2 * 128, D_CH, ST], BF16, kind="ExternalInput")
    mask = nc.dram_tensor("mask", [128, 256], BF16, kind="ExternalInput")
    # weights pre-tiled on host: w[p, i, m] = W[m, i*128+p]
    wqT = nc.dram_tensor("wqT", [128, D_CH, DH], BF16, kind="ExternalInput")
    wkT = nc.dram_tensor("wkT", [128, D_CH, DH], BF16, kind="ExternalInput")
    wvT = nc.dram_tensor("wvT", [128, D_CH, DH], BF16, kind="ExternalInput")
    bq = nc.dram_tensor("bq", [DH, 1], F32, kind="ExternalInput")
    bk = nc.dram_tensor("bk", [DH, 1], F32, kind="ExternalInput")
    bv = nc.dram_tensor("bv", [DH, 1], F32, kind="ExternalInput")
    out = nc.dram_tensor("out", [SQ, DH], BF16, kind="ExternalOutput")
    # comm bounce: per stage [128, 1024] = [K^T 512 | V(pos-major) 512]
    cc_in = nc.dram_tensor("cc_in", [2 * 128, 2 * ST], BF16, kind="Internal")
    cc_out = nc.dram_tensor("cc_out", [2 * 256, 2 * ST], BF16, kind="Internal")

    with tile.TileContext(nc) as tc:
        with (
            tc.tile_pool(name="weights", bufs=1) as wpool,
            tc.tile_pool(name="persist", bufs=1) as persist,
        ):
            w_sb = {}
            for name in ("q", "k", "v"):
                w_sb[name] = wpool.tile([128, D_CH, DH], BF16, tag=f"w{name}",
                                        name=f"w{name}")
            W_EXT = {"q": wqT, "k": wkT, "v": wvT}

            def w_half(g):
                ss = np.s_[:, g * 16:(g + 1) * 16, :]
                for name in ("k", "q", "v"):
                    nc.sync.dma_start(out=w_sb[name][ss], in_=W_EXT[name][ss])
            b_sb = {}
            for name, ext in (("q", bq), ("k", bk), ("v", bv)):
                t = wpool.tile([DH, 1], F32, tag=f"b{name}")
                b_sb[name] = t
            mk = persist.tile([128, 256], BF16, tag="mk")
            nc.gpsimd.dma_start(out=mk[:], in_=mask[:])

            # rank-indexed: rank0 keys at [0:1024], rank1 at [1024:2048]
            kt_sb = persist.tile([DH, S], BF16, tag="kt")
            v_sb = persist.tile([128, 2 * NT, DH], BF16, tag="v")
            qt_sb = persist.tile([DH, SQ], BF16, tag="qt")   # own Q^T
            x_st = [persist.tile([128, D_CH, ST], BF16, tag=f"x{st}",
                                 name=f"x{st}") for st in (0, 1)]
            kst = [persist.tile([DH, ST], BF16, tag=f"kst{st}", name=f"kst{st}")
                   for st in (0, 1)]
            vst = [persist.tile([DH, ST], BF16, tag=f"vst{st}", name=f"vst{st}")
                   for st in (0, 1)]
            vtr = [persist.tile([128, 4, DH], BF16, tag=f"vtr{st}", name=f"vtr{st}")
                   for st in (0, 1)]

            # --- phase 1: projections in two column-stages + pair AllGather ---
            with tc.tile_pool(name="ppsum1", bufs=1, space="PSUM") as pp1:
                acc = {}
                for tag in ("pk0", "pq0", "pv0", "pk1", "pv1", "pq1"):
                    acc[tag] = pp1.tile([DH, ST], F32, tag=tag, name=tag)

                def x_piece(st, i0, n):
                    nc.sync.dma_start(out=x_st[st][:, i0:i0 + n, :],
                                      in_=xT[st * 128:(st + 1) * 128, i0:i0 + n, :])

                # DMA issue order is the sync-queue service order: x pieces
                # and weight halves ONLY (everything else lives on other
                # queues so the x stream is never blocked behind a wait)
                x_piece(0, 0, 4)
                w_half(0)
                for i0, n in ((4, 8), (12, 8), (20, 8), (28, 4)):
                    x_piece(0, i0, n)
                w_half(1)
                x_piece(1, 0, 4)
                for i0, n in ((4, 8), (12, 8), (20, 8), (28, 4)):
                    x_piece(1, i0, n)
                for name, ext in (("q", bq), ("k", bk), ("v", bv)):
                    nc.sync.dma_start(out=b_sb[name][:], in_=ext[:])

                def stage_mms(st, with_q):
                    for i in range(D_CH):
                        stt = dict(start=(i == 0), stop=(i == D_CH - 1))
                        nc.tensor.matmul(acc[f"pk{st}"][:], lhsT=w_sb["k"][:, i, :],
                                         rhs=x_st[st][:, i, :], **stt)
                        if with_q:
                            nc.tensor.matmul(acc[f"pq{st}"][:], lhsT=w_sb["q"][:, i, :],
                                             rhs=x_st[st][:, i, :], **stt)
                        nc.tensor.matmul(acc[f"pv{st}"][:], lhsT=w_sb["v"][:, i, :],
                                         rhs=x_st[st][:, i, :], **stt)

                def stage_comm(st):
                    o = st * ST
                    nc.vector.tensor_scalar_add(kst[st][:], acc[f"pk{st}"][:],
                                                b_sb["k"][:])
                    nc.vector.tensor_scalar_add(vst[st][:], acc[f"pv{st}"][:],
                                                b_sb["v"][:])
                    nc.scalar.dma_start_transpose(out=vtr[st][:], in_=vst[st][:])
                    nc.gpsimd.dma_start(out=cc_in[st * 128:(st + 1) * 128, 0:ST],
                                        in_=kst[st][:])
                    nc.gpsimd.dma_start(out=cc_in[st * 128:(st + 1) * 128, ST:2 * ST],
                                        in_=vtr[st][:])
                    nc.gpsimd.collective_compute(
                        "AllGather", mybir.AluOpType.bypass,
                        replica_groups=PAIRS,
                        ins=[cc_in[st * 128:(st + 1) * 128, :].opt()],
                        outs=[cc_out[st * 256:(st + 1) * 256, :].opt()],
                    )
                    r0 = st * 256
                    r1 = st * 256 + 128
                    nc.gpsimd.dma_start(out=kt_sb[:, o:o + ST],
                                        in_=cc_out[r0:r0 + 128, 0:ST])
                    nc.gpsimd.dma_start(out=kt_sb[:, SQ + o:SQ + o + ST],
                                        in_=cc_out[r1:r1 + 128, 0:ST])
                    nc.gpsimd.dma_start(out=v_sb[:, st * 4:st * 4 + 4, :],
                                        in_=cc_out[r0:r0 + 128, ST:2 * ST])
                    nc.gpsimd.dma_start(out=v_sb[:, NT + st * 4:NT + st * 4 + 4, :],
                                        in_=cc_out[r1:r1 + 128, ST:2 * ST])

                # stage A: K, Q, V interleaved per chunk
                stage_mms(0, with_q=True)
                nc.vector.tensor_scalar_add(qt_sb[:, 0:ST], acc["pq0"][:],
                                            b_sb["q"][:])
                stage_comm(0)
                # stage B: K, V only (Q deferred so gather B starts early)
                stage_mms(1, with_q=False)
                stage_comm(1)
                # Q of stage B
                for i in range(D_CH):
                    nc.tensor.matmul(acc["pq1"][:], lhsT=w_sb["q"][:, i, :],
                                     rhs=x_st[1][:, i, :],
                                     start=(i == 0), stop=(i == D_CH - 1))
                nc.vector.tensor_scalar_add(qt_sb[:, ST:2 * ST], acc["pq1"][:],
                                            b_sb["q"][:])

            # --- phase 2: per-tile softmax + PV (baseline machinery) ---
            with (
                tc.tile_pool(name="lg_psum", bufs=4, space="PSUM") as lg,
                tc.tile_pool(name="o_psum", bufs=2, space="PSUM") as opool,
                tc.tile_pool(name="lm_sb", bufs=1) as lmpool,
                tc.tile_pool(name="wt_sb", bufs=1) as wtpool,
                tc.tile_pool(name="stats", bufs=12) as stat,
                tc.tile_pool(name="out_sb", bufs=2) as ospool,
            ):
                pv_args = {}

                def softmax_stage(j):
                    e = j + 1            # chunks per rank range
                    w = e * 128          # cols per rank range
                    qsl = np.s_[:, j * 128:(j + 1) * 128]

                    # Only the LAST chunk of each rank range carries mask
                    # values (host bakes h into mk): rank0 last = triangle
                    # (h=0) or all-0 (h=1); rank1 last = all--1e9 (h=0) or
                    # triangle (h=1). Other chunks are pure past: plain
                    # psum f32 -> bf16 rounding.
                    lmt = lmpool.tile([128, 2 * w], BF16, tag=f"lm{j}")
                    for base, off, mcol in ((0, 0, 0), (SQ, w, 128)):
                        for g0 in range(0, w, 512):
                            gw = min(512, w - g0)
                            pg = lg.tile([128, 512], F32, tag="pg")
                            nc.tensor.matmul(pg[:, :gw], lhsT=qt_sb[qsl],
                                             rhs=kt_sb[:, base + g0:base + g0 + gw],
                                             start=True, stop=True)
                            last = g0 + gw == w
                            cp = gw - 128 if last else gw
                            if cp:
                                nc.vector.tensor_copy(
                                    lmt[:, off + g0:off + g0 + cp], pg[:, :cp])
                            if last:
                                nc.vector.tensor_add(
                                    lmt[:, off + g0 + cp:off + g0 + gw],
                                    pg[:, cp:gw],
                                    mk[:, mcol:mcol + 128])
                    negmax = stat.tile([128, 1], F32, tag="negmax")
                    nc.vector.reduce_max(out=negmax[:], in_=lmt[:, :2 * w],
                                         axis=mybir.AxisListType.X, negate=True)
                    w_t = lmpool.tile([128, 2 * w], BF16, tag=f"w{j}")
                    wt_t = wtpool.tile([128, 2 * e, 128], BF16, tag=f"wt{j}")
                    sumexp = stat.tile([128, 1], F32, tag="sumexp")
                    nc.scalar.activation(
                        out=w_t[:, :2 * w], in_=lmt[:, :2 * w],
                        func=mybir.ActivationFunctionType.Exp,
                        bias=negmax[:], scale=1.0, accum_out=sumexp[:])
                    nc.sync.dma_start_transpose(out=wt_t[:, :2 * e, :],
                                                in_=w_t[:, :2 * w])
                    pv_args[j] = (wt_t, sumexp, e)

                def pv_stage(j):
                    wt_t, sumexp, e = pv_args.pop(j)
                    rsum = stat.tile([128, 1], F32, tag="rsum")
                    nc.vector.reciprocal(rsum[:], sumexp[:])
                    po = opool.tile([128, DH], F32, tag="po")
                    for c in range(2 * e):
                        vc = c if c < e else NT + (c - e)
                        nc.tensor.matmul(po[:], lhsT=wt_t[:, c, :], rhs=v_sb[:, vc, :],
                                         start=(c == 0), stop=(c == 2 * e - 1))
                    o_sb = ospool.tile([128, DH], BF16, tag="o")
                    nc.vector.tensor_scalar_mul(o_sb[:], po[:], rsum[:])
                    nc.sync.dma_start(out=out[j * 128:(j + 1) * 128, :], in_=o_sb[:])

                # small tiles (need only gather A) first -- they cover
                # gather B's latency; then large tiles largest-first so the
                # last softmax chains hide under other tiles' PE work
                softmax_stage(3)
                softmax_stage(2)
                softmax_stage(1)
                softmax_stage(0)
                pv_stage(3)
                pv_stage(2)
                pv_stage(1)
                pv_stage(0)
                softmax_stage(7)
                softmax_stage(6)
                softmax_stage(5)
                softmax_stage(4)
                pv_stage(7)
                pv_stage(6)
                pv_stage(5)
                pv_stage(4)

    nc.finalize()
    return nc


def shard_inputs(x, attn_mask, Wq, bq, Wk, bk, Wv, bv):
    """Host-side shard prep. Returns in_maps for cores 0..7."""
    bf = ml_dtypes.bfloat16
    xb = np.asarray(x).astype(bf)                   # cast first, like the reference
    mask_f = np.asarray(attn_mask)

    def tile_w(W):
        WT = np.asarray(W).astype(bf).T.reshape(D_CH, 128, DH)
        return np.ascontiguousarray(WT.transpose(1, 0, 2))

    wqt, wkt, wvt = tile_w(Wq), tile_w(Wk), tile_w(Wv)
    bqc = np.asarray(bq).astype(bf).astype(np.float32).reshape(DH, 1)
    bkc = np.asarray(bk).astype(bf).astype(np.float32).reshape(DH, 1)
    bvc = np.asarray(bv).astype(bf).astype(np.float32).reshape(DH, 1)

    tri = mask_f[:128, :128].astype(bf)       # causal triangle (0/-1e9)
    zeros = np.zeros((128, 128), dtype=bf)
    neg = np.full((128, 128), -1e9, dtype=np.float32).astype(bf)

    in_maps = []
    for c in range(N_CORES):
        b, h = divmod(c, 2)
        own = np.concatenate([np.arange(t * 128, (t + 1) * 128)
                              for t in range(h, 16, 2)])
        xcols = xb[b][own]                            # [1024, D]
        # [st, s, i, p] -> [st, p, i, s], flatten stage into rows
        xa = np.ascontiguousarray(
            xcols.reshape(2, ST, D_CH, 128).transpose(0, 3, 2, 1)
        ).reshape(2 * 128, D_CH, ST)
        if h == 0:
            msk = np.concatenate([tri, neg], axis=1)       # rank0 diag, rank1 future
        else:
            msk = np.concatenate([zeros, tri], axis=1)     # rank0 past, rank1 diag
        in_maps.append({
            "xT": xa, "mask": np.ascontiguousarray(msk),
            "wqT": wqt, "wkT": wkt, "wvT": wvt,
            "bq": bqc, "bk": bkc, "bv": bvc,
        })
    return in_maps


_NC_CACHE = {}


def kernel(x, attn_mask, Wq, bq, Wk, bk, Wv, bv):
    if "nc" not in _NC_CACHE:
        _NC_CACHE["nc"] = build_nc()
    nc = _NC_CACHE["nc"]
    in_maps = shard_inputs(x, attn_mask, Wq, bq, Wk, bk, Wv, bv)
    res = run_bass_kernel_spmd(nc, in_maps, list(range(N_CORES)))
    out = np.empty((B, S, DH), dtype=ml_dtypes.bfloat16)
    for c in range(N_CORES):
        b, h = divmod(c, 2)
        for j in range(NT):
            t = 2 * j + h
            out[b, t * 128:(t + 1) * 128, :] = res.results[c]["out"][j * 128:(j + 1) * 128]
    return out


# revision 15
# speedup vs baseline: 1.1502x; 1.0494x over previous
"""Trainium2 Bass kernel for a single attention head (B=4, S=2048, D=4096, DH=128).

Sharding: 8 cores = (batch b, half h), pair (2b, 2b+1) shares batch b.
Core (b,h) owns the INTERLEAVED tile set {t : t % 2 == h} (8 tiles of 128
positions) -- both its q rows AND its K/V key chunks. Each core projects
Q, K, V only for its OWN 1024 columns of x (half the work of replicating
K/V), then the pair exchanges K/V halves with a pairwise AllGather.

Rank-symmetric layout (same NEFF on every core): K^T / V go through a
DRAM bounce + AllGather and are read back RANK-indexed (rank0 = even
tiles, rank1 = odd tiles), so no access pattern depends on h. The
causal structure relative to rank0/rank1 is baked into the host-provided
mask block (triangle / zeros / -1e9), exactly like the baseline's
own/other mask trick.

Phases:
  1: two column-stages (own cols 0:512, 512:1024). Per stage, stream x
     chunks and accumulate K (+Q in stage A) and V. Stage drains feed a
     bounce buffer; AllGather per stage; readback fills kt_sb/v_sb both
     rank halves. Q of stage B is deferred until after stage B's K/V so
     the second gather starts as early as possible.
  2: per local q-tile j (global 2j+h): logits over rank0 range
     [0:(j+1)*128] and rank1 range [1024:1024+(j+1)*128], mask on the
     last chunk of each range -> rowmax -> exp (accum rowsum) -> W^T via
     DMA transpose -> PV -> scale -> out. Small tiles (gather-A-only)
     run first to cover gather B's latency; large tiles largest-first.
"""

import numpy as np
import ml_dtypes

import concourse.bass as bass
import concourse.tile as tile
from concourse import bacc, mybir
from concourse.bass_utils import run_bass_kernel_spmd

B, S, D, DH = 4, 2048, 4096, 128
SQ = S // 2          # own q rows / own key cols per core
ST = 512             # columns per stage
N_CORES = 8
D_CH = D // 128      # 32 contraction chunks
NT = 8               # local q tiles (slots)
PAIRS = [[0, 1], [2, 3], [4, 5], [6, 7]]

BF16 = mybir.dt.bfloat16
F32 = mybir.dt.float32


def build_nc():
    nc = bacc.Bacc(None)

    # x own columns, stage-major: xT[st*128+p, i, s] = x[b, col(st*512+s), i*128+p]
    xT = nc.dram_tensor("xT", [2 * 128, D_CH, ST], BF16, kind="ExternalInput")
    mask = nc.dram_tensor("mask", [128, 256], BF16, kind="ExternalInput")
    # weights pre-tiled on host: w[p, i, m] = W[m, i*128+p]
    wqT = nc.dram_tensor("wqT", [128, D_CH, DH], BF16, kind="ExternalInput")
    wkT = nc.dram_tensor("wkT", [128, D_CH, DH], BF16, kind="ExternalInput")
    wvT = nc.dram_tensor("wvT", [128, D_CH, DH], BF16, kind="ExternalInput")
    bq = nc.dram_tensor("bq", [DH, 1], F32, kind="ExternalInput")
    bk = nc.dram_tensor("bk", [DH, 1], F32, kind="ExternalInput")
    bv = nc.dram_tensor("bv", [DH, 1], F32, kind="ExternalInput")
    out = nc.dram_tensor("out", [SQ, DH], BF16, kind="ExternalOutput")
    # comm bounce: per stage [128, 1024] = [K^T 512 | V(pos-major) 512]
    cc_in = nc.dram_tensor("cc_in", [2 * 128, 2 * ST], BF16, kind="Internal")
    cc_out = nc.dram_tensor("cc_out", [2 * 256, 2 * ST], BF16, kind="Internal")
    # tiny warmup collective: absorbs first-call TOPSP/communicator latency
    ccw_in = nc.dram_tensor("ccw_in", [1, 64], BF16, kind="Internal")
    ccw_out = nc.dram_tensor("ccw_out", [2, 64], BF16, kind="Internal")

    with tile.TileContext(nc) as tc:
        with (
            tc.tile_pool(name="weights", bufs=1) as wpool,
            tc.tile_pool(name="persist", bufs=1) as persist,
        ):
            w_sb = {}
            for name in ("q", "k", "v"):
                w_sb[name] = wpool.tile([128, D_CH, DH], BF16, tag=f"w{name}",
                                        name=f"w{name}")
            W_EXT = {"q": wqT, "k": wkT, "v": wvT}

            def w_half(g):
                ss = np.s_[:, g * 16:(g + 1) * 16, :]
                for name in ("k", "q", "v"):
                    nc.sync.dma_start(out=w_sb[name][ss], in_=W_EXT[name][ss])
            b_sb = {}
            for name, ext in (("q", bq), ("k", bk), ("v", bv)):
                t = wpool.tile([DH, 1], F32, tag=f"b{name}")
                b_sb[name] = t
            nc.gpsimd.dma_start(out=ccw_in.ap(), in_=mask[0:1, 0:64])
            nc.gpsimd.collective_compute(
                "AllGather", mybir.AluOpType.bypass,
                replica_groups=PAIRS,
                ins=[ccw_in.ap().opt()], outs=[ccw_out.ap().opt()],
            )
            mk = persist.tile([128, 256], BF16, tag="mk")
            nc.gpsimd.dma_start(out=mk[:], in_=mask[:])

            # rank-indexed: rank0 keys at [0:1024], rank1 at [1024:2048]
            kt_sb = persist.tile([DH, S], BF16, tag="kt")
            v_sb = persist.tile([128, 2 * NT, DH], BF16, tag="v")
            qt_sb = persist.tile([DH, SQ], BF16, tag="qt")   # own Q^T
            x_st = [persist.tile([128, D_CH, ST], BF16, tag=f"x{st}",
                                 name=f"x{st}") for st in (0, 1)]
            kst = [persist.tile([DH, ST], BF16, tag=f"kst{st}", name=f"kst{st}")
                   for st in (0, 1)]
            vst = [persist.tile([DH, ST], BF16, tag=f"vst{st}", name=f"vst{st}")
                   for st in (0, 1)]
            vtr = [persist.tile([128, 4, DH], BF16, tag=f"vtr{st}", name=f"vtr{st}")
                   for st in (0, 1)]

            # --- phase 1: projections in two column-stages + pair AllGather ---
            with tc.tile_pool(name="ppsum1", bufs=1, space="PSUM") as pp1:
                acc = {}
                for tag in ("pk0", "pq0", "pv0", "pk1", "pv1", "pq1"):
                    acc[tag] = pp1.tile([DH, ST], F32, tag=tag, name=tag)

                def x_piece(st, i0, n):
                    nc.sync.dma_start(out=x_st[st][:, i0:i0 + n, :],
                                      in_=xT[st * 128:(st + 1) * 128, i0:i0 + n, :])

                # DMA issue order is the sync-queue service order: graduated
                # x pieces + k/v weights early, q weights late (Q is
                # deferred), biases last. Nothing else rides this queue in
                # phase 1 so the x stream is never blocked behind a wait.
                def wsl(name, i0, n):
                    ss = np.s_[:, i0:i0 + n, :]
                    nc.sync.dma_start(out=w_sb[name][ss], in_=W_EXT[name][ss])

                x_piece(0, 0, 2)
                wsl("k", 0, 4)
                wsl("v", 0, 4)
                x_piece(0, 2, 2)
                wsl("k", 4, 12)
                wsl("v", 4, 12)
                x_piece(0, 4, 4)
                x_piece(0, 8, 8)
                wsl("k", 16, 16)
                wsl("v", 16, 16)
                x_piece(0, 16, 8)
                x_piece(0, 24, 8)
                wsl("q", 0, 16)
                wsl("q", 16, 16)
                for i0, n in ((0, 8), (8, 8), (16, 8), (24, 8)):
                    x_piece(1, i0, n)
                for name, ext in (("q", bq), ("k", bk), ("v", bv)):
                    nc.sync.dma_start(out=b_sb[name][:], in_=ext[:])

                def stage_mms(st):
                    for i in range(D_CH):
                        stt = dict(start=(i == 0), stop=(i == D_CH - 1))
                        nc.tensor.matmul(acc[f"pk{st}"][:], lhsT=w_sb["k"][:, i, :],
                                         rhs=x_st[st][:, i, :], **stt)
                        nc.tensor.matmul(acc[f"pv{st}"][:], lhsT=w_sb["v"][:, i, :],
                                         rhs=x_st[st][:, i, :], **stt)

                def stage_comm(st):
                    nc.vector.tensor_scalar_add(kst[st][:], acc[f"pk{st}"][:],
                                                b_sb["k"][:])
                    nc.vector.tensor_scalar_add(vst[st][:], acc[f"pv{st}"][:],
                                                b_sb["v"][:])
                    nc.scalar.dma_start_transpose(out=vtr[st][:], in_=vst[st][:])
                    nc.gpsimd.dma_start(out=cc_in[st * 128:(st + 1) * 128, 0:ST],
                                        in_=kst[st][:])
                    nc.gpsimd.dma_start(out=cc_in[st * 128:(st + 1) * 128, ST:2 * ST],
                                        in_=vtr[st][:])
                    nc.gpsimd.collective_compute(
                        "AllGather", mybir.AluOpType.bypass,
                        replica_groups=PAIRS,
                        ins=[cc_in[st * 128:(st + 1) * 128, :].opt()],
                        outs=[cc_out[st * 256:(st + 1) * 256, :].opt()],
                    )

                def stage_readback(st):
                    o = st * ST
                    r0 = st * 256
                    r1 = st * 256 + 128
                    nc.gpsimd.dma_start(out=kt_sb[:, o:o + ST],
                                        in_=cc_out[r0:r0 + 128, 0:ST])
                    nc.gpsimd.dma_start(out=kt_sb[:, SQ + o:SQ + o + ST],
                                        in_=cc_out[r1:r1 + 128, 0:ST])
                    nc.gpsimd.dma_start(out=v_sb[:, st * 4:st * 4 + 4, :],
                                        in_=cc_out[r0:r0 + 128, ST:2 * ST])
                    nc.gpsimd.dma_start(out=v_sb[:, NT + st * 4:NT + st * 4 + 4, :],
                                        in_=cc_out[r1:r1 + 128, ST:2 * ST])

                # K/V only, both stages; Q deferred so both gathers trigger
                # as early as possible and Q covers their latency
                stage_mms(0)
                stage_comm(0)
                stage_mms(1)
                stage_comm(1)
                stage_readback(0)
                stage_readback(1)
                for st in (0, 1):
                    for i in range(D_CH):
                        nc.tensor.matmul(acc[f"pq{st}"][:], lhsT=w_sb["q"][:, i, :],
                                         rhs=x_st[st][:, i, :],
                                         start=(i == 0), stop=(i == D_CH - 1))
                    nc.vector.tensor_scalar_add(qt_sb[:, st * ST:(st + 1) * ST],
                                                acc[f"pq{st}"][:], b_sb["q"][:])

            # --- phase 2: per-tile softmax + PV (baseline machinery) ---
            with (
                tc.tile_pool(name="lg_psum", bufs=4, space="PSUM") as lg,
                tc.tile_pool(name="o_psum", bufs=2, space="PSUM") as opool,
                tc.tile_pool(name="lm_sb", bufs=1) as lmpool,
                tc.tile_pool(name="wt_sb", bufs=1) as wtpool,
                tc.tile_pool(name="stats", bufs=12) as stat,
                tc.tile_pool(name="out_sb", bufs=2) as ospool,
            ):
                pv_args = {}

                def softmax_stage(j):
                    e = j + 1            # chunks per rank range
                    w = e * 128          # cols per rank range
                    qsl = np.s_[:, j * 128:(j + 1) * 128]

                    # Only the LAST chunk of each rank range carries mask
                    # values (host bakes h into mk): rank0 last = triangle
                    # (h=0) or all-0 (h=1); rank1 last = all--1e9 (h=0) or
                    # triangle (h=1). Other chunks are pure past: plain
                    # psum f32 -> bf16 rounding.
                    lmt = lmpool.tile([128, 2 * w], BF16, tag=f"lm{j}")
                    for base, off, mcol in ((0, 0, 0), (SQ, w, 128)):
                        for g0 in range(0, w, 512):
                            gw = min(512, w - g0)
                            pg = lg.tile([128, 512], F32, tag="pg")
                            nc.tensor.matmul(pg[:, :gw], lhsT=qt_sb[qsl],
                                             rhs=kt_sb[:, base + g0:base + g0 + gw],
                                             start=True, stop=True)
                            last = g0 + gw == w
                            cp = gw - 128 if last else gw
                            if cp:
                                nc.vector.tensor_copy(
                                    lmt[:, off + g0:off + g0 + cp], pg[:, :cp])
                            if last:
                                nc.vector.tensor_add(
                                    lmt[:, off + g0 + cp:off + g0 + gw],
                                    pg[:, cp:gw],
                                    mk[:, mcol:mcol + 128])
                    negmax = stat.tile([128, 1], F32, tag="negmax")
                    nc.vector.reduce_max(out=negmax[:], in_=lmt[:, :2 * w],
                                         axis=mybir.AxisListType.X, negate=True)
                    w_t = lmpool.tile([128, 2 * w], BF16, tag=f"w{j}")
                    wt_t = wtpool.tile([128, 2 * e, 128], BF16, tag=f"wt{j}")
                    sumexp = stat.tile([128, 1], F32, tag="sumexp")
                    nc.scalar.activation(
                        out=w_t[:, :2 * w], in_=lmt[:, :2 * w],
                        func=mybir.ActivationFunctionType.Exp,
                        bias=negmax[:], scale=1.0, accum_out=sumexp[:])
                    nc.sync.dma_start_transpose(out=wt_t[:, :2 * e, :],
                                                in_=w_t[:, :2 * w])
                    pv_args[j] = (wt_t, sumexp, e)

                def pv_stage(j):
                    wt_t, sumexp, e = pv_args.pop(j)
                    rsum = stat.tile([128, 1], F32, tag="rsum")
                    nc.vector.reciprocal(rsum[:], sumexp[:])
                    po = opool.tile([128, DH], F32, tag="po")
                    for c in range(2 * e):
                        vc = c if c < e else NT + (c - e)
                        nc.tensor.matmul(po[:], lhsT=wt_t[:, c, :], rhs=v_sb[:, vc, :],
                                         start=(c == 0), stop=(c == 2 * e - 1))
                    o_sb = ospool.tile([128, DH], BF16, tag="o")
                    nc.vector.tensor_scalar_mul(o_sb[:], po[:], rsum[:])
                    nc.sync.dma_start(out=out[j * 128:(j + 1) * 128, :], in_=o_sb[:])

                # small tiles (need only gather A) first; big-tile logits
                # fill the small tiles' softmax-chain latency; largest-first
                # so the longest chains start earliest
                softmax_stage(3)
                softmax_stage(2)
                softmax_stage(1)
                softmax_stage(0)
                softmax_stage(7)
                softmax_stage(6)
                pv_stage(3)
                pv_stage(2)
                pv_stage(1)
                pv_stage(0)
                softmax_stage(5)
                softmax_stage(4)
                pv_stage(7)
                pv_stage(6)
                pv_stage(5)
                pv_stage(4)

    nc.finalize()
    return nc


def shard_inputs(x, attn_mask, Wq, bq, Wk, bk, Wv, bv):
    """Host-side shard prep. Returns in_maps for cores 0..7."""
    bf = ml_dtypes.bfloat16
    xb = np.asarray(x).astype(bf)                   # cast first, like the reference
    mask_f = np.asarray(attn_mask)

    def tile_w(W):
        WT = np.asarray(W).astype(bf).T.reshape(D_CH, 128, DH)
        return np.ascontiguousarray(WT.transpose(1, 0, 2))

    wqt, wkt, wvt = tile_w(Wq), tile_w(Wk), tile_w(Wv)
    bqc = np.asarray(bq).astype(bf).astype(np.float32).reshape(DH, 1)
    bkc = np.asarray(bk).astype(bf).astype(np.float32).reshape(DH, 1)
    bvc = np.asarray(bv).astype(bf).astype(np.float32).reshape(DH, 1)

    tri = mask_f[:128, :128].astype(bf)       # causal triangle (0/-1e9)
    zeros = np.zeros((128, 128), dtype=bf)
    neg = np.full((128, 128), -1e9, dtype=np.float32).astype(bf)

    in_maps = []
    for c in range(N_CORES):
        b, h = divmod(c, 2)
        own = np.concatenate([np.arange(t * 128, (t + 1) * 128)
                              for t in range(h, 16, 2)])
        xcols = xb[b][own]                            # [1024, D]
        # [st, s, i, p] -> [st, p, i, s], flatten stage into rows
        xa = np.ascontiguousarray(
            xcols.reshape(2, ST, D_CH, 128).transpose(0, 3, 2, 1)
        ).reshape(2 * 128, D_CH, ST)
        if h == 0:
            msk = np.concatenate([tri, neg], axis=1)       # rank0 diag, rank1 future
        else:
            msk = np.concatenate([zeros, tri], axis=1)     # rank0 past, rank1 diag
        in_maps.append({
            "xT": xa, "mask": np.ascontiguousarray(msk),
            "wqT": wqt, "wkT": wkt, "wvT": wvt,
            "bq": bqc, "bk": bkc, "bv": bvc,
        })
    return in_maps


_NC_CACHE = {}


def kernel(x, attn_mask, Wq, bq, Wk, bk, Wv, bv):
    if "nc" not in _NC_CACHE:
        _NC_CACHE["nc"] = build_nc()
    nc = _NC_CACHE["nc"]
    in_maps = shard_inputs(x, attn_mask, Wq, bq, Wk, bk, Wv, bv)
    res = run_bass_kernel_spmd(nc, in_maps, list(range(N_CORES)))
    out = np.empty((B, S, DH), dtype=ml_dtypes.bfloat16)
    for c in range(N_CORES):
        b, h = divmod(c, 2)
        for j in range(NT):
            t = 2 * j + h
            out[b, t * 128:(t + 1) * 128, :] = res.results[c]["out"][j * 128:(j + 1) * 128]
    return out
